# revision 15
# baseline (speedup 1.0000x reference)
"""Trainium2 Bass kernel for nn_Aggregator_32959579030024.

Computes out[n, d] = curr_emb[n, 0, d] + sum_k alpha[n, k, 0] * msg[n, k, d]
for N=100000, K=32, D=128 (fp32), sharded over 8 NeuronCores on the node dim.

Math: per tile of `tile_n` nodes, SBUF partition p holds msg row 128*g + p of
the tile (g = 4-node group, tile_n/4 groups/tile); each group's 128 partitions
are the (node-in-group m, neighbor k) rows of 4 nodes. A block-diagonal alpha
tile [128, 4] per group (alpha[4g+m, k] at partition 32m+k, column m) is the
moving operand of a matmul whose stationary operand is the msg slice
[128, 128]:

    psum[d, m] += sum_{p=(m,k)} msg[(m,k), d] * alphadiag[(m,k), m]
               =  sum_k alpha[node, k] * msg[node, k, d]

PSUM holds the tile transposed as [d, node]. DVE adds host-transposed curr
during PSUM evacuation; the d-major result is DMA'd out and the host
transposes it back.

Precision/perf: fp32 matmuls on trn2 stream weights at ~38 Gelem/s (PE-bound),
so the host splits msg exactly into bf16 hi + lo (same DMA bytes) and alpha
into a + b; each group runs 2 bf16 matmuls accumulating in fp32 PSUM:
    mm1: h x [a|b] -> psum[.., 0:2, :],  mm2: l x [a|0] -> same region
which computes m*alpha ~= h*a + h*b + l*a (dropped l*b term ~2^-18 relative);
the two PSUM halves are summed by DVE during evacuation. Both matmuls of a
group write the identical 8-column PSUM region (mismatched accumulation-group
regions hung the device).

DMA: msg, compact (a,b,a) alpha, and curr (bit-cast to bf16 pairs) are
host-packed into ONE contiguous per-tile block ([128 partitions, ~34KB] at
tile_n=256) so each tile needs a single read DMA of full-size packets —
measured pure-DMA rate here is ~415 GB/s vs ~282 GB/s when small strided DMAs
fragment the queues. Alpha is expanded to block-diagonal on-chip by 4 DVE
copies into persistent pre-zeroed buffers; output writes are batched 7 tiles
per DMA. The node dim is zero-padded to a tile multiple so tiles are uniform.
"""

import numpy as np

N, K, D = 100000, 32, 128
CORES = 8
NS = N // CORES              # 12500 nodes per shard
TILE_N = 256                 # nodes per tile (kernel default)
MSG_BUFS = 4
OUT_BATCH = 7                # tiles per batched output DMA

_cache = {}


def _dims(ns, tile_n):
    nt = (ns + tile_n - 1) // tile_n
    ng = tile_n // 4
    mg = ng * 2 * D          # bf16 elems of msg hi/lo per partition
    ag = ng * 3              # bf16 elems of compact (a,b,a) alpha per partition
    cg = 2 * tile_n          # bf16 elems (bit-cast fp32 curr) per partition
    return nt, ng, mg, ag, mg + ag + cg


def build_program(ns=NS, tile_n=TILE_N, msg_bufs=MSG_BUFS, ob=OUT_BATCH):
    import concourse.bacc as bacc
    import concourse.mybir as mybir
    import concourse.tile as tile

    nt, ng, mg, ag, F = _dims(ns, tile_n)
    if nt % ob:
        ob = next(d for d in (7, 5, 4, 3, 2, 1) if nt % d == 0)
    nc = bacc.Bacc("TRN2", target_bir_lowering=False, debug=False)
    f32 = mybir.dt.float32
    bf16 = mybir.dt.bfloat16
    u16 = mybir.dt.uint16
    inp = nc.dram_tensor("inp", [nt, 128, F], u16, kind="ExternalInput")
    assert nt % ob == 0, (nt, ob)
    out = nc.dram_tensor("out", [nt // ob, D, ob * tile_n], f32, kind="ExternalOutput")

    with tile.TileContext(nc) as tc:
        with (
            tc.tile_pool(name="inpool", bufs=msg_bufs) as inpool,
            tc.tile_pool(name="alpool", bufs=1) as alpool,
            tc.tile_pool(name="outp", bufs=4 if tile_n <= 256 else 2) as outp,
            tc.tile_pool(name="psump", bufs=4 if tile_n <= 256 else 3,
                         space="PSUM") as psump,
        ):
            # Persistent block-diag alpha buffers: zeroed once; each tile
            # rewrites only the (fixed) diagonal slots, so off-diagonal
            # zeros and the fourth (zero) quad survive across tiles.
            AB = 3
            al_bufs = [
                alpool.tile([128, ng, 4, 4], bf16, name=f"albuf{i}",
                            tag=f"al{i}")
                for i in range(AB)
            ]
            for ab in al_bufs:
                nc.vector.memset(ab[:], 0.0)
            for t in range(nt):
                it = inpool.tile([128, F], u16, tag="inp")
                nc.sync.dma_start(it[:], inp[t])
                msgv = it[:, :mg].bitcast(bf16).rearrange("p (g two d) -> p g two d", two=2, d=D)
                acv = it[:, mg:mg + ag].bitcast(bf16).rearrange(
                    "p (g three) -> p g three", three=3
                )
                curv = it[:, mg + ag:].bitcast(f32)

                al_t = al_bufs[t % AB]
                for m in range(4):
                    nc.vector.tensor_copy(
                        al_t[32 * m:32 * (m + 1), :, 0:3, m],
                        acv[32 * m:32 * (m + 1), :, :],
                    )

                # psum holds [d, g, hl, m]: hl=0 accumulates h*a + l*a,
                # hl=1 holds h*b; the two halves are summed during evac.
                ps = psump.tile([128, ng, 2, 4], f32, tag="ps")
                for g in range(ng):
                    h = msgv[:, g, 0, :]
                    lo = msgv[:, g, 1, :]
                    ab = al_t[:, g, 0:2, :]    # [a | b]
                    az = al_t[:, g, 2:4, :]    # [a | 0]
                    nc.tensor.matmul(ps[:, g, :, :], h, ab, start=True, stop=False)
                    nc.tensor.matmul(ps[:, g, :, :], lo, az, start=False, stop=True)

                if t % ob == 0:
                    ot = outp.tile([128, ob * tile_n], f32, tag="out")
                osl = ot[:, (t % ob) * tile_n:(t % ob + 1) * tile_n].rearrange(
                    "p (g m) -> p g m", m=4
                )
                cur3 = curv.rearrange("p (g m) -> p g m", m=4)
                nc.vector.tensor_add(osl, ps[:, :, 0, :], cur3)
                nc.vector.tensor_add(osl, osl, ps[:, :, 1, :])
                if t % ob == ob - 1:
                    nc.sync.dma_start(out[t // ob], ot[:])

    nc.compile()
    return nc


def _split_bf16(x):
    import ml_dtypes

    hi = x.astype(ml_dtypes.bfloat16)
    lo = (x - hi.astype(np.float32)).astype(ml_dtypes.bfloat16)
    return hi, lo


def make_in_maps(curr_emb, alpha, msg, ns=NS, tile_n=TILE_N):
    import ml_dtypes

    bf16 = ml_dtypes.bfloat16
    curr_emb = np.asarray(curr_emb, dtype=np.float32)
    alpha = np.asarray(alpha, dtype=np.float32)
    msg = np.asarray(msg, dtype=np.float32)
    n = curr_emb.shape[0]
    cores = n // ns
    nt, ng, mg, ag, F = _dims(ns, tile_n)
    nsp = nt * tile_n
    pad = nsp - ns
    in_maps = []
    for c in range(cores):
        sl = slice(c * ns, (c + 1) * ns)

        m = msg[sl].reshape(ns * K, D)
        if pad:
            m = np.concatenate([m, np.zeros((pad * K, D), np.float32)], axis=0)
        m_hi, m_lo = _split_bf16(m)
        # rows (128g + p) -> [nt, p, g, hl, d], flattened per partition
        m_hi = m_hi.reshape(nt, ng, 128, D).transpose(0, 2, 1, 3)
        m_lo = m_lo.reshape(nt, ng, 128, D).transpose(0, 2, 1, 3)
        msg_part = np.stack([m_hi, m_lo], axis=3).reshape(nt, 128, mg)

        a = alpha[sl, :, 0]
        if pad:
            a = np.concatenate([a, np.zeros((pad, K), np.float32)], axis=0)
        a_hi, a_lo = _split_bf16(a)
        # Compact (a, b, a) per diag slot: aldg[t, 32m+k, g, q] = alpha
        # quads for node 4g+m, neighbor k (expanded to block-diag on-chip).
        aldg = np.zeros((nt, 4, K, ng, 3), dtype=bf16)
        ah = a_hi.reshape(nt, ng, 4, K)
        al = a_lo.reshape(nt, ng, 4, K)
        for mm in range(4):
            aht = ah[:, :, mm, :].transpose(0, 2, 1)
            aldg[:, mm, :, :, 0] = aht
            aldg[:, mm, :, :, 1] = al[:, :, mm, :].transpose(0, 2, 1)
            aldg[:, mm, :, :, 2] = aht
        al_part = aldg.reshape(nt, 128, ag)

        cur = curr_emb[sl, 0, :]
        if pad:
            cur = np.concatenate([cur, np.zeros((pad, D), np.float32)], axis=0)
        # currT[d, tile nodes] bit-cast to bf16 pairs: [nt, 128(d), 2*tile_n]
        curT = np.ascontiguousarray(cur.T)  # [D, nsp]
        cur_part = (
            curT.reshape(D, nt, tile_n).transpose(1, 0, 2)
            .copy().view(bf16).reshape(nt, 128, 2 * tile_n)
        )

        combined = np.concatenate(
            [msg_part.view(np.uint16), al_part.view(np.uint16),
             cur_part.view(np.uint16)], axis=2
        )
        in_maps.append({"inp": np.ascontiguousarray(combined)})
    return in_maps


def gather_out(per_core_outs, ns=NS, tile_n=TILE_N):
    shards = []
    for o in per_core_outs:
        nb = o.shape[0] * o.shape[2]  # total padded nodes
        # [ntg, D, ob*tile_n] -> [ntg, ob*tile_n, D] -> [nsp, D] -> [ns, D]
        shards.append(o.transpose(0, 2, 1).reshape(nb, D)[:ns])
    return np.concatenate(shards, axis=0)


def kernel(curr_emb, alpha, msg):
    from concourse.bass_utils import run_bass_kernel_spmd

    if "nc" not in _cache:
        _cache["nc"] = build_program()
    nc = _cache["nc"]
    in_maps = make_in_maps(curr_emb, alpha, msg)
    # The accelerator occasionally reports NRT_EXEC_UNIT_UNRECOVERABLE on a
    # run (intermittent; same program passes on retry). Reset the jax/PJRT
    # backend and retry before giving up.
    last = None
    for attempt in range(3):
        try:
            res = run_bass_kernel_spmd(nc, in_maps, list(range(CORES)))
            return gather_out([res.results[c]["out"] for c in range(CORES)])
        except Exception as e:  # noqa: BLE001
            last = e
            try:
                import jax

                jax.clear_caches()
                jax.extend.backend.clear_backends()
            except Exception:
                pass
    raise last
